# revision 6
# baseline (speedup 1.0000x reference)
"""MoE (dense-act-dense, top-4 of 8 experts) Trainium2 kernel.

Strategy (expert-parallel, host-side dispatch):
  - The forward combine weight is exactly 1.0 (straight-through gate trick in
    the reference), so out[n] = sum_{e in top4(n)} expert_e(x[n]).
  - Host computes the tiny gate matmul + top-4 routing (0.05% of FLOPs) and
    dispatches tokens: core e receives the tokens routed to expert e
    (capacity-padded), plus expert e's weights. This is the sharding step.
  - Each of the 8 cores runs a dense 2-layer MLP (relu between) on its tokens:
      h = relu(w1[e] @ x) ; y = w2[e] @ h
    as two chained fp32r GEMMs (fp32 data, FP22 multiply, fp32 accumulate).
  - Host scatter-adds per-expert outputs back (weight 1.0 per selection).

Per-core device layouts (everything pre-transposed on host for contiguous DMA):
  xT  [D, C] f32r : routed tokens, transposed
  w1t [D, H] f32r : w1[e].T
  w2t [H, O] f32r : w2[e].T
  yT  [O, C] f32  : expert output, transposed

Schedule notes (v2):
  - Weights are DMAed in 128-wide column slices (separate tiles) so GEMM
    chains start as soon as their slice lands instead of after the full 8MB.
  - DMA emission order on the sync queue is hand-tuned: x0, w1 slices, x1,
    w2[0:8], x2, w2[8:16], x3, ... so the PE's program-order needs roughly
    track the FIFO queue's delivery order during the ~26MB startup stream.
  - GEMM2(t) is emitted one tile behind GEMM1(t+1) (depth-1 software
    pipeline) to give the PE GEMM1 work while w2 is still streaming in.
  - y drains: PSUM -> SBUF copy on vector, store DMA issued on scalar, so the
    sync queue (x + weights, latency-critical) is never blocked behind them.
"""

import numpy as np
from contextlib import ExitStack

import concourse.bass as bass
import concourse.tile as tile
from concourse import bacc, mybir
from concourse import bass_utils

F32 = mybir.dt.float32
F32R = mybir.dt.float32r
P = 128

TOP_K = 4
D, H, O, E = 2048, 1024, 2048, 8
NT = 416          # token tile (matmul moving free dim); >=256 keeps fp32r at 1 cyc/row

_NC_CACHE = {}


def build_expert_kernel(C):
    """Per-core program: dense [C, D] @ [D, H] -> relu -> @ [H, O] in fp32r."""
    DC, HC, OC = D // P, H // P, O // P
    NTILES = C // NT
    assert C % NT == 0
    nc = bacc.Bacc("TRN2", target_bir_lowering=False, debug=False, num_devices=E)
    xT = nc.dram_tensor("xT", [D, C], F32R, kind="ExternalInput").ap()
    w1t = nc.dram_tensor("w1t", [D, H], F32R, kind="ExternalInput").ap()
    w2t = nc.dram_tensor("w2t", [H, O], F32R, kind="ExternalInput").ap()
    yT = nc.dram_tensor("yT", [O, C], F32, kind="ExternalOutput").ap()

    with tile.TileContext(nc) as tc, ExitStack() as ctx:
        wpool = ctx.enter_context(tc.tile_pool(name="w", bufs=1))
        xpool = ctx.enter_context(tc.tile_pool(name="x", bufs=2))
        hpool = ctx.enter_context(tc.tile_pool(name="h", bufs=2))
        ypool = ctx.enter_context(tc.tile_pool(name="y", bufs=2))
        ps1 = ctx.enter_context(tc.tile_pool(name="ps1", bufs=2, space="PSUM"))
        ps2 = ctx.enter_context(tc.tile_pool(name="ps2", bufs=4, space="PSUM"))

        x_tiles = {}

        def dma_x(t):
            x_t = xpool.tile([P, DC, NT], F32R, name="x_t")
            nc.sync.dma_start(
                x_t[:],
                xT[:, t * NT:(t + 1) * NT].rearrange("(dc p) n -> p dc n", p=P),
            )
            x_tiles[t] = x_t

        # --- startup DMA stream, hand-ordered for the FIFO queue ---
        dma_x(0)
        w1s = []
        for hc in range(HC):
            w = wpool.tile([P, DC, P], F32R, name=f"w1s{hc}")
            nc.sync.dma_start(
                w[:],
                w1t[:, hc * P:(hc + 1) * P].rearrange("(dc p) h -> p dc h", p=P),
            )
            w1s.append(w)
        if NTILES > 1:
            dma_x(1)
        w2s = []

        def dma_w2(oc):
            w = wpool.tile([P, HC, P], F32R, name=f"w2s{oc}")
            nc.sync.dma_start(
                w[:],
                w2t[:, oc * P:(oc + 1) * P].rearrange("(hc p) o -> p hc o", p=P),
            )
            w2s.append(w)

        for oc in range(OC // 2):
            dma_w2(oc)

        def gemm1(t):
            x_t = x_tiles.pop(t)
            h_t = hpool.tile([P, HC, NT], F32R, name="h_t")
            for hc in range(HC):
                ph = ps1.tile([P, NT], F32, name="ph")
                for dc in range(DC):
                    nc.tensor.matmul(
                        ph[:], w1s[hc][:, dc, :], x_t[:, dc, :],
                        start=(dc == 0), stop=(dc == DC - 1),
                    )
                nc.scalar.activation(
                    h_t[:, hc, :], ph[:], mybir.ActivationFunctionType.Relu
                )
            return h_t

        def gemm2(t, h_t):
            for oc in range(OC):
                po = ps2.tile([P, NT], F32, name="po")
                for hc in range(HC):
                    nc.tensor.matmul(
                        po[:], w2s[oc][:, hc, :], h_t[:, hc, :],
                        start=(hc == 0), stop=(hc == HC - 1),
                    )
                # y staging is tight on SBUF: drain each PSUM chain in two
                # half-width buffers so copies/stores double-buffer in ~1.7KB.
                HNT = NT // 2
                for s in range(2):
                    y_t = ypool.tile([P, HNT], F32, name="y_t")
                    nc.vector.tensor_copy(y_t[:], po[:, s * HNT:(s + 1) * HNT])
                    nc.scalar.dma_start(
                        yT[oc * P:(oc + 1) * P,
                           t * NT + s * HNT:t * NT + (s + 1) * HNT],
                        y_t[:],
                    )

        # --- depth-1 software-pipelined main loop ---
        h_tiles = {}
        for t in range(NTILES):
            if t + 1 < NTILES and t >= 1:
                dma_x(t + 1)
            h_tiles[t] = gemm1(t)
            if t == 1:
                for oc in range(OC // 2, OC):
                    dma_w2(oc)
            if t >= 1:
                gemm2(t - 1, h_tiles.pop(t - 1))
        gemm2(NTILES - 1, h_tiles.pop(NTILES - 1))
    nc.compile()
    return nc


def _route(xt, wg):
    """Host-side gate + top-4. Gap between 4th/5th gate values is ~3e-5 for
    this distribution, far above fp32 matmul noise, so fp32 reproduces the
    reference top-k set exactly."""
    gate = xt @ wg  # [N, E] fp32
    top4 = np.argpartition(-gate, TOP_K - 1, axis=1)[:, :TOP_K]  # set, unordered
    return top4


def kernel(x, wg, w1, w2, _want_results=False, _run_kwargs=None):
    B, S, Dx = x.shape
    N = B * S
    xt = np.ascontiguousarray(x.reshape(N, Dx))
    top4 = _route(xt, wg)

    # token lists per expert
    sel = np.zeros((N, E), dtype=bool)
    np.put_along_axis(sel, top4, True, axis=1)
    tokens = [np.nonzero(sel[:, e])[0] for e in range(E)]
    counts = np.array([len(t) for t in tokens])
    CAP = int(-(-counts.max() // NT) * NT)
    CAP = max(CAP, 2 * NT)

    if CAP not in _NC_CACHE:
        _NC_CACHE[CAP] = build_expert_kernel(CAP)
    nc = _NC_CACHE[CAP]

    in_maps = []
    for e in range(E):
        xe = np.zeros((CAP, Dx), dtype=np.float32)
        xe[:counts[e]] = xt[tokens[e]]
        in_maps.append({
            "xT": np.ascontiguousarray(xe.T),
            "w1t": np.ascontiguousarray(w1[e].T),
            "w2t": np.ascontiguousarray(w2[e].T),
        })

    res = bass_utils.run_bass_kernel_spmd(
        nc, in_maps, core_ids=list(range(E)), **(_run_kwargs or {})
    )

    out = np.zeros((N, O), dtype=np.float32)
    for e in range(E):
        out[tokens[e]] += res.results[e]["yT"].T[:counts[e]]
    out = out.reshape(B, S, O)
    if _want_results:
        return out, res
    return out


# revision 7
# speedup vs baseline: 1.2239x; 1.2239x over previous
"""MoE (dense-act-dense, top-4 of 8 experts) Trainium2 kernel.

Strategy (expert-parallel, host-side dispatch):
  - The forward combine weight is exactly 1.0 (straight-through gate trick in
    the reference), so out[n] = sum_{e in top4(n)} expert_e(x[n]).
  - Host computes the tiny gate matmul + top-4 routing (0.05% of FLOPs) and
    dispatches tokens: core e receives the tokens routed to expert e
    (capacity-padded), plus expert e's weights. This is the sharding step.
  - Each of the 8 cores runs a dense 2-layer MLP (relu between) on its tokens:
      h = relu(w1[e] @ x) ; y = w2[e] @ h
    as two chained fp32r GEMMs (fp32 data, FP22 multiply, fp32 accumulate).
  - Host scatter-adds per-expert outputs back (weight 1.0 per selection).

Per-core device layouts (everything pre-transposed on host for contiguous DMA):
  xT  [D, C] f32r : routed tokens, transposed
  w1t [D, H] f32r : w1[e].T
  w2t [H, O] f32r : w2[e].T
  yT  [O, C] f32  : expert output, transposed

Schedule notes (v2):
  - Weights are DMAed in 128-wide column slices (separate tiles) so GEMM
    chains start as soon as their slice lands instead of after the full 8MB.
  - DMA emission order on the sync queue is hand-tuned: x0, w1 slices, x1,
    w2[0:8], x2, w2[8:16], x3, ... so the PE's program-order needs roughly
    track the FIFO queue's delivery order during the ~26MB startup stream.
  - GEMM2(t) is emitted one tile behind GEMM1(t+1) (depth-1 software
    pipeline) to give the PE GEMM1 work while w2 is still streaming in.
  - y drains: PSUM -> SBUF copy on vector, store DMA issued on scalar, so the
    sync queue (x + weights, latency-critical) is never blocked behind them.
"""

import numpy as np
from contextlib import ExitStack

import concourse.bass as bass
import concourse.tile as tile
from concourse import bacc, mybir
from concourse import bass_utils

F32 = mybir.dt.float32
F32R = mybir.dt.float32r
P = 128

TOP_K = 4
D, H, O, E = 2048, 1024, 2048, 8
NT = 384          # token tile (matmul moving free dim); >=256 keeps fp32r at 1 cyc/row

_NC_CACHE = {}


def build_expert_kernel(C):
    """Per-core program: dense [C, D] @ [D, H] -> relu -> @ [H, O] in fp32r."""
    DC, HC, OC = D // P, H // P, O // P
    NTILES = C // NT
    assert C % NT == 0
    nc = bacc.Bacc("TRN2", target_bir_lowering=False, debug=False, num_devices=E)
    xT = nc.dram_tensor("xT", [D, C], F32R, kind="ExternalInput").ap()
    w1t = nc.dram_tensor("w1t", [D, H], F32R, kind="ExternalInput").ap()
    w2t = nc.dram_tensor("w2t", [H, O], F32R, kind="ExternalInput").ap()
    yT = nc.dram_tensor("yT", [O, C], F32, kind="ExternalOutput").ap()

    with tile.TileContext(nc) as tc, ExitStack() as ctx:
        wpool = ctx.enter_context(tc.tile_pool(name="w", bufs=1))
        xpool = ctx.enter_context(tc.tile_pool(name="x", bufs=2))
        hpool = ctx.enter_context(tc.tile_pool(name="h", bufs=2))
        ypool = ctx.enter_context(tc.tile_pool(name="y", bufs=4))
        ps1 = ctx.enter_context(tc.tile_pool(name="ps1", bufs=2, space="PSUM"))
        ps2 = ctx.enter_context(tc.tile_pool(name="ps2", bufs=4, space="PSUM"))

        x_tiles = {}

        def dma_x(t):
            x_t = xpool.tile([P, DC, NT], F32R, name="x_t")
            nc.sync.dma_start(
                x_t[:],
                xT[:, t * NT:(t + 1) * NT].rearrange("(dc p) n -> p dc n", p=P),
            )
            x_tiles[t] = x_t

        # --- startup DMA stream, hand-ordered for the FIFO queue ---
        dma_x(0)
        w1s = []
        for hc in range(HC):
            w = wpool.tile([P, DC, P], F32R, name=f"w1s{hc}")
            nc.sync.dma_start(
                w[:],
                w1t[:, hc * P:(hc + 1) * P].rearrange("(dc p) h -> p dc h", p=P),
            )
            w1s.append(w)
        if NTILES > 1:
            dma_x(1)
        w2s = []

        def dma_w2(oc):
            w = wpool.tile([P, HC, P], F32R, name=f"w2s{oc}")
            nc.sync.dma_start(
                w[:],
                w2t[:, oc * P:(oc + 1) * P].rearrange("(hc p) o -> p hc o", p=P),
            )
            w2s.append(w)

        for oc in range(OC // 2):
            dma_w2(oc)

        def gemm1(t):
            x_t = x_tiles.pop(t)
            h_t = hpool.tile([P, HC, NT], F32R, name="h_t")
            for hc in range(HC):
                ph = ps1.tile([P, NT], F32, name="ph")
                for dc in range(DC):
                    nc.tensor.matmul(
                        ph[:], w1s[hc][:, dc, :], x_t[:, dc, :],
                        start=(dc == 0), stop=(dc == DC - 1),
                    )
                nc.scalar.activation(
                    h_t[:, hc, :], ph[:], mybir.ActivationFunctionType.Relu
                )
            return h_t

        def gemm2(t, h_t):
            for oc in range(OC):
                po = ps2.tile([P, NT], F32, name="po")
                for hc in range(HC):
                    nc.tensor.matmul(
                        po[:], w2s[oc][:, hc, :], h_t[:, hc, :],
                        start=(hc == 0), stop=(hc == HC - 1),
                    )
                y_t = ypool.tile([P, NT], F32, name="y_t")
                nc.vector.tensor_copy(y_t[:], po[:])
                nc.scalar.dma_start(
                    yT[oc * P:(oc + 1) * P, t * NT:(t + 1) * NT], y_t[:]
                )

        # --- depth-1 software-pipelined main loop ---
        h_tiles = {}
        for t in range(NTILES):
            if t + 1 < NTILES and t >= 1:
                dma_x(t + 1)
            h_tiles[t] = gemm1(t)
            if t == 1:
                for oc in range(OC // 2, OC):
                    dma_w2(oc)
            if t >= 1:
                gemm2(t - 1, h_tiles.pop(t - 1))
        gemm2(NTILES - 1, h_tiles.pop(NTILES - 1))
    nc.compile()
    return nc


def _route(xt, wg):
    """Host-side gate + top-4. Gap between 4th/5th gate values is ~3e-5 for
    this distribution, far above fp32 matmul noise, so fp32 reproduces the
    reference top-k set exactly."""
    gate = xt @ wg  # [N, E] fp32
    top4 = np.argpartition(-gate, TOP_K - 1, axis=1)[:, :TOP_K]  # set, unordered
    return top4


def kernel(x, wg, w1, w2, _want_results=False, _run_kwargs=None):
    B, S, Dx = x.shape
    N = B * S
    xt = np.ascontiguousarray(x.reshape(N, Dx))
    top4 = _route(xt, wg)

    # token lists per expert
    sel = np.zeros((N, E), dtype=bool)
    np.put_along_axis(sel, top4, True, axis=1)
    tokens = [np.nonzero(sel[:, e])[0] for e in range(E)]
    counts = np.array([len(t) for t in tokens])
    CAP = int(-(-counts.max() // NT) * NT)
    CAP = max(CAP, 2 * NT)

    if CAP not in _NC_CACHE:
        _NC_CACHE[CAP] = build_expert_kernel(CAP)
    nc = _NC_CACHE[CAP]

    in_maps = []
    for e in range(E):
        xe = np.zeros((CAP, Dx), dtype=np.float32)
        xe[:counts[e]] = xt[tokens[e]]
        in_maps.append({
            "xT": np.ascontiguousarray(xe.T),
            "w1t": np.ascontiguousarray(w1[e].T),
            "w2t": np.ascontiguousarray(w2[e].T),
        })

    res = bass_utils.run_bass_kernel_spmd(
        nc, in_maps, core_ids=list(range(E)), **(_run_kwargs or {})
    )

    out = np.zeros((N, O), dtype=np.float32)
    for e in range(E):
        out[tokens[e]] += res.results[e]["yT"].T[:counts[e]]
    out = out.reshape(B, S, O)
    if _want_results:
        return out, res
    return out


# revision 14
# speedup vs baseline: 1.2282x; 1.0035x over previous
"""MoE (dense-act-dense, top-4 of 8 experts) Trainium2 kernel.

Strategy (expert-parallel, host-side dispatch):
  - The forward combine weight is exactly 1.0 (straight-through gate trick in
    the reference), so out[n] = sum_{e in top4(n)} expert_e(x[n]).
  - Host computes the tiny gate matmul + top-4 routing (0.05% of FLOPs) and
    dispatches tokens: core e receives the tokens routed to expert e
    (capacity-padded), plus expert e's weights. This is the sharding step.
  - Each of the 8 cores runs a dense 2-layer MLP (relu between) on its tokens:
      h = relu(w1[e] @ x) ; y = w2[e] @ h
    as two chained fp32r GEMMs (fp32 data, FP22 multiply, fp32 accumulate).
  - Host scatter-adds per-expert outputs back (weight 1.0 per selection).

Per-core device layouts (everything pre-transposed on host for contiguous DMA):
  xT  [D, C] f32r : routed tokens, transposed
  w1t [D, H] f32r : w1[e].T
  w2t [H, O] f32r : w2[e].T
  yT  [O, C] f32  : expert output, transposed

Schedule notes (v2):
  - Weights are DMAed in 128-wide column slices (separate tiles) so GEMM
    chains start as soon as their slice lands instead of after the full 8MB.
  - DMA emission order on the sync queue is hand-tuned: x0, w1 slices, x1,
    w2[0:8], x2, w2[8:16], x3, ... so the PE's program-order needs roughly
    track the FIFO queue's delivery order during the ~26MB startup stream.
  - GEMM2(t) is emitted one tile behind GEMM1(t+1) (depth-1 software
    pipeline) to give the PE GEMM1 work while w2 is still streaming in.
  - y drains: PSUM -> SBUF copy on vector, store DMA issued on scalar, so the
    sync queue (x + weights, latency-critical) is never blocked behind them.
"""

import numpy as np
from contextlib import ExitStack

import concourse.bass as bass
import concourse.tile as tile
from concourse import bacc, mybir
from concourse import bass_utils

F32 = mybir.dt.float32
F32R = mybir.dt.float32r
P = 128

TOP_K = 4
D, H, O, E = 2048, 1024, 2048, 8
_NC_CACHE = {}


def _tile_widths(C, target):
    """Split C tokens (padded to even) into even tiles of near-equal width in
    [256, 512]. Even widths are an fp32r ISA requirement; >=256 keeps fp32r at
    1 cycle/row; wider tiles amortize the fixed ~32-cycle per-matmul bubble."""
    C = max(C + (C % 2), 256)
    C2 = C // 2
    ntiles = min(-(-C // target), C2 // 128)
    base = C2 // ntiles
    rem = C2 - base * ntiles
    widths = [2 * (base + 1)] * rem + [2 * base] * (ntiles - rem)
    widths.sort(reverse=True)
    assert sum(widths) == C and all(256 <= w <= 512 and w % 2 == 0 for w in widths)
    return widths


def build_expert_kernel(C, target):
    """Per-core program: dense [C, D] @ [D, H] -> relu -> @ [H, O] in fp32r."""
    DC, HC, OC = D // P, H // P, O // P
    widths = _tile_widths(C, target)
    starts = [sum(widths[:i]) for i in range(len(widths))]
    NTILES = len(widths)
    NTMAX = max(widths)
    nc = bacc.Bacc("TRN2", target_bir_lowering=False, debug=False, num_devices=E)
    xT = nc.dram_tensor("xT", [D, C], F32R, kind="ExternalInput").ap()
    w1t = nc.dram_tensor("w1t", [D, H], F32R, kind="ExternalInput").ap()
    w2t = nc.dram_tensor("w2t", [H, O], F32R, kind="ExternalInput").ap()
    yT = nc.dram_tensor("yT", [O, C], F32, kind="ExternalOutput").ap()

    with tile.TileContext(nc) as tc, ExitStack() as ctx:
        wpool = ctx.enter_context(tc.tile_pool(name="w", bufs=1))
        xpool = ctx.enter_context(tc.tile_pool(name="x", bufs=2))
        hpool = ctx.enter_context(tc.tile_pool(name="h", bufs=1))
        ypool = ctx.enter_context(tc.tile_pool(name="y", bufs=4))
        ps1 = ctx.enter_context(tc.tile_pool(name="ps1", bufs=3, space="PSUM"))
        ps2 = ctx.enter_context(tc.tile_pool(name="ps2", bufs=5, space="PSUM"))

        x_tiles = {}

        def dma_x(t):
            w_t = widths[t]
            x_t = xpool.tile([P, DC, NTMAX], F32R, name="x_t")[:, :, :w_t]
            nc.sync.dma_start(
                x_t[:],
                xT[:, starts[t]:starts[t] + w_t].rearrange("(dc p) n -> p dc n", p=P),
            )
            x_tiles[t] = x_t

        # --- startup DMA stream, hand-ordered for the FIFO queue ---
        dma_x(0)
        w1s = []
        for hc in range(HC):
            w = wpool.tile([P, DC, P], F32R, name=f"w1s{hc}")
            nc.sync.dma_start(
                w[:],
                w1t[:, hc * P:(hc + 1) * P].rearrange("(dc p) h -> p dc h", p=P),
            )
            w1s.append(w)
        if NTILES > 1:
            dma_x(1)
        w2s = []

        def dma_w2(oc):
            w = wpool.tile([P, HC, P], F32R, name=f"w2s{oc}")
            nc.sync.dma_start(
                w[:],
                w2t[:, oc * P:(oc + 1) * P].rearrange("(hc p) o -> p hc o", p=P),
            )
            w2s.append(w)

        for oc in range(OC):
            dma_w2(oc)

        def gemm1(t):
            w_t = widths[t]
            x_t = x_tiles.pop(t)
            h_t = hpool.tile([P, HC, NTMAX], F32R, name="h_t")[:, :, :w_t]
            for hc in range(HC):
                ph = ps1.tile([P, NTMAX], F32, name="ph")[:, :w_t]
                for dc in range(DC):
                    nc.tensor.matmul(
                        ph[:], w1s[hc][:, dc, :], x_t[:, dc, :],
                        start=(dc == 0), stop=(dc == DC - 1),
                    )
                nc.scalar.activation(
                    h_t[:, hc, :], ph[:], mybir.ActivationFunctionType.Relu
                )
            return h_t

        def gemm2(t, h_t):
            w_t = widths[t]
            for oc in range(OC):
                po = ps2.tile([P, NTMAX], F32, name="po")[:, :w_t]
                for hc in range(HC):
                    nc.tensor.matmul(
                        po[:], w2s[oc][:, hc, :], h_t[:, hc, :],
                        start=(hc == 0), stop=(hc == HC - 1),
                    )
                y_t = ypool.tile([P, NTMAX], F32, name="y_t")[:, :w_t]
                nc.vector.tensor_copy(y_t[:], po[:])
                nc.scalar.dma_start(
                    yT[oc * P:(oc + 1) * P, starts[t]:starts[t] + w_t], y_t[:]
                )

        # --- main loop (depth-0; x prefetch one tile ahead) ---
        for t in range(NTILES):
            if t + 1 < NTILES and t >= 1:
                dma_x(t + 1)
            h_t = gemm1(t)
            gemm2(t, h_t)
    nc.compile()
    return nc


def _route(xt, wg):
    """Host-side gate + top-4. Gap between 4th/5th gate values is ~3e-5 for
    this distribution, far above fp32 matmul noise, so fp32 reproduces the
    reference top-k set exactly."""
    gate = xt @ wg  # [N, E] fp32
    top4 = np.argpartition(-gate, TOP_K - 1, axis=1)[:, :TOP_K]  # set, unordered
    return top4


def kernel(x, wg, w1, w2, _want_results=False, _run_kwargs=None):
    B, S, Dx = x.shape
    N = B * S
    xt = np.ascontiguousarray(x.reshape(N, Dx))
    top4 = _route(xt, wg)

    # token lists per expert
    sel = np.zeros((N, E), dtype=bool)
    np.put_along_axis(sel, top4, True, axis=1)
    tokens = [np.nonzero(sel[:, e])[0] for e in range(E)]
    counts = np.array([len(t) for t in tokens])
    CAP = max(int(counts.max()), 256)
    CAP += CAP % 2

    if CAP not in _NC_CACHE:
        # Wider tiles amortize the per-matmul bubble best, but the widest
        # config cuts SBUF very close — fall back to narrower tiles if the
        # allocator rejects it.
        last_err = None
        for target in (461, 416, 384):
            try:
                _NC_CACHE[CAP] = build_expert_kernel(CAP, target)
                break
            except ValueError as err:  # SBUF pool allocation failure
                last_err = err
        else:
            raise last_err
    nc = _NC_CACHE[CAP]

    in_maps = []
    for e in range(E):
        xe = np.zeros((CAP, Dx), dtype=np.float32)
        xe[:counts[e]] = xt[tokens[e]]
        in_maps.append({
            "xT": np.ascontiguousarray(xe.T),
            "w1t": np.ascontiguousarray(w1[e].T),
            "w2t": np.ascontiguousarray(w2[e].T),
        })

    res = bass_utils.run_bass_kernel_spmd(
        nc, in_maps, core_ids=list(range(E)), **(_run_kwargs or {})
    )

    out = np.zeros((N, O), dtype=np.float32)
    for e in range(E):
        out[tokens[e]] += res.results[e]["yT"].T[:counts[e]]
    out = out.reshape(B, S, O)
    if _want_results:
        return out, res
    return out
